# revision 28
# baseline (speedup 1.0000x reference)
"""Trainium2 Bass kernel for nn_DMLoss_61942018343083 (Chamfer-style polygon
matching loss, retrieval_knn).

Sharding: data-parallel over batch B=32 across 8 NeuronCores (4 batches/core).
Each core computes partial sums into a [128, 12] output tile; the host combines
them into the scalar loss.

v2 design (vs the fp32-matmul v1):

pred2gt (argmin over 5120 interp points for each of 512 preds):
  * Ranking key v[p, (t,i)] = -d^2(p, interp(t,i)) + |p-256|^2 computed on the
    PE as a K=14 bf16 matmul per (pred-chunk, t): coordinates are recentered by
    -256 and split hi/lo into bf16 pairs (p ~ p_hi + p_lo), so each product
    p*r = p_hi*r_hi + p_hi*r_lo + p_lo*r_hi is exact to ~1 unit (lo*lo
    dropped).  bf16 matmuls run at 1 cycle/column vs fp32's 4.
  * All 14-row operand blocks are HOST-PREPARED (numpy) and DMA'd in; the
    3-per-tile packing at base partitions 0/32/64 satisfies the PE constraint
    that lhsT/rhs share a base partition in {0,32,64}.
  * t=0..5 accumulate in a 6-bank PSUM tile, reduced with one vector
    tensor_reduce(max) over a strided [128, 512, 6] view; t=6..9 drain via
    scalar ACTIVATE(Identity, bias=-|p|^2) to self-scaled bf16 and merge with
    3 vector TT(max) ops.  The [128,5120] key is never materialized.
  * MAX8/FIND_INDEX8 on the final [128,512] column-max give the best segment
    i* per pred; ONE indirect DMA per chunk gathers that segment's 10 interp
    points (host-prepped i-major table [512, 10*2]); exact fp32 refine over
    the 10 candidates picks the true nearest (CPU-sim: rel err 1.4e-4).

gt2pred (argmin over 512 preds for each of 512 gts):
  * Same trick, orientation flipped: K=8 bf16 matmul per gt-chunk
    (psum = 2*g.p - |p|^2), scalar drain with bias -|g|^2 -> bf16 -d^2 key,
    top-1 via MAX8/FIND_INDEX8, gather pred row, masked abs-diff partials.
"""

import os
import sys

for _p in ("/opt/trn_rl_repo", "/root/.axon_site/_ro/trn_rl_repo"):
    if os.path.isdir(_p) and _p not in sys.path:
        sys.path.insert(0, _p)

import numpy as np
import ml_dtypes

bfloat16 = ml_dtypes.bfloat16

import concourse.bass as bass
import concourse.bacc as bacc
import concourse.mybir as mybir
from concourse.bass import IndirectOffsetOnAxis
from concourse.bass_utils import run_bass_kernel_spmd
from concourse.tile import TileContext

F32 = mybir.dt.float32
BF16 = mybir.dt.bfloat16
U32 = mybir.dt.uint32
AF = mybir.ActivationFunctionType
ALU = mybir.AluOpType
AX = mybir.AxisListType

B, NP, NG, T = 32, 512, 512, 10
NCORES = 8
BLOC = B // NCORES          # 4 batches per core
NCH = NP // 128             # 4 chunks of 128 preds / 128 gts
CEN = np.float32(256.0)     # recentering shift
NRED = 6                    # t-banks reduced on vector; T-NRED drained on scalar
K14 = 14                    # pred2gt contraction rows
K8 = 8                      # gt2pred contraction rows


def _split_hi_lo(x):
    x = np.asarray(x, dtype=np.float32)
    hi = x.astype(bfloat16)
    lo = (x - hi.astype(np.float32)).astype(bfloat16)
    return hi, lo


def host_prep(ini_pred_poly, gt_polys):
    """Build all matmul operands / tables for one core's BLOC batches."""
    f = np.float32
    a = (np.arange(T, dtype=np.float32) / f(T)).astype(np.float32)   # t/10
    b_ = (f(1.0) - a).astype(np.float32)

    ini = np.asarray(ini_pred_poly, dtype=np.float32)   # [BLOC, NP, 2]
    gt = np.asarray(gt_polys, dtype=np.float32)         # [BLOC, NG, 2]
    gtr = np.roll(gt, 1, axis=1)

    pc = ini - CEN
    gc = gt - CEN
    gcr = np.roll(gc, 1, axis=1)

    pxh, pxl = _split_hi_lo(pc[:, :, 0])
    pyh, pyl = _split_hi_lo(pc[:, :, 1])
    m1 = np.full_like(pxh, -1.0)

    # ---- pred2gt projection operands ----
    # Q[p,i] = (p - g_{i-1}) . s_i    (s = g_i - g_{i-1})
    # R[p,i] = -|p - g_{i-1}|^2  =  2 p.g_ - |g_|^2 - |p|^2   (centered coords)
    pp = (pc * pc).sum(-1).astype(np.float32)               # |p-256|^2 [BLOC, NP]
    s = (gc - gcr).astype(np.float32)
    len2 = (s * s).sum(-1).astype(np.float32)
    inv10 = np.where(len2 > 1e-6, (f(10.0) / len2).astype(np.float32),
                     np.float32(0.0)).astype(np.float32)
    negalpha = (-(len2 / f(100.0))).astype(np.float32)
    gs_ = (gcr * s).sum(-1).astype(np.float32)
    ug_ = (gcr * gcr).sum(-1).astype(np.float32)

    one = np.ones_like(pxh, dtype=np.float32)
    sxh, sxl = _split_hi_lo((s[:, :, 0] * inv10).astype(np.float32))
    syh, syl = _split_hi_lo((s[:, :, 1] * inv10).astype(np.float32))
    gsh, gsl = _split_hi_lo((gs_ * inv10).astype(np.float32))
    lhsQ = np.stack([pxh, pxh, pxl, pyh, pyh, pyl, m1, m1],
                    axis=1).astype(bfloat16)                  # [BLOC, 8, NP]
    rhsQ = np.stack([sxh, sxl, sxh, syh, syl, syh, gsh, gsl],
                    axis=1).astype(bfloat16)                  # [BLOC, 8, NG]

    g2rxh, g2rxl = _split_hi_lo(f(2.0) * gcr[:, :, 0])
    g2ryh, g2ryl = _split_hi_lo(f(2.0) * gcr[:, :, 1])
    ugh, ugl = _split_hi_lo(ug_)
    npph, nppl = _split_hi_lo(-pp)
    oneb = one.astype(bfloat16)
    lhsR = np.stack([pxh, pxh, pxl, pyh, pyh, pyl, m1, m1, npph, nppl],
                    axis=1).astype(bfloat16)                  # [BLOC, 10, NP]
    rhsR = np.stack([g2rxh, g2rxl, g2rxh, g2ryh, g2ryl, g2ryh, ugh, ugl,
                     oneb, oneb], axis=1).astype(bfloat16)    # [BLOC, 10, NG]
    # per-column broadcast tile (replicated across 128 partitions host-side)
    len2_b = np.broadcast_to(len2[:, None, :], (BLOC, 128, NG)).copy()

    # ---- interp table, i-major wide rows [BLOC, NG, T*2] fp32 (bit-exact ref math)
    itabw = np.empty((BLOC, NG, T, 2), dtype=np.float32)
    for t in range(T):
        itabw[:, :, t, :] = (gt * a[t]).astype(np.float32) + (gtr * (f(1.0) - a[t])).astype(np.float32)
    itabw = itabw.reshape(BLOC, NG, T * 2)

    # ---- gt2pred: lhs rows [g2xh,g2xh,g2xl,g2yh,g2yh,g2yl,m1,m1] over gts
    g2xh, g2xl = _split_hi_lo(f(2.0) * gc[:, :, 0])
    g2yh, g2yl = _split_hi_lo(f(2.0) * gc[:, :, 1])
    m1g = np.full_like(g2xh, -1.0)
    gtl8 = np.stack([g2xh, g2xh, g2xl, g2yh, g2yh, g2yl, m1g, m1g],
                    axis=1).astype(bfloat16)                 # [BLOC, 8, NG]
    pph, ppl = _split_hi_lo(pp)
    prhs8 = np.stack([pxh, pxl, pxh, pyh, pyl, pyh, pph, ppl],
                     axis=1).astype(bfloat16)                # [BLOC, 8, NP]
    ug = (gc * gc).sum(-1).astype(np.float32)                # |g-256|^2 [BLOC, NG]
    uneg = (-ug).reshape(BLOC, NCH, 128).transpose(0, 2, 1).copy()

    return dict(lhsQ=lhsQ, rhsQ=rhsQ, lhsR=lhsR, rhsR=rhsR, len2_b=len2_b,
                itabw=itabw, gtl8=gtl8, prhs8=prhs8, uneg=uneg)


def build_nc():
    nc = bacc.Bacc()

    ini = nc.dram_tensor("ini_pred_poly", [BLOC, NP, 2], F32, kind="ExternalInput")
    pred2 = nc.dram_tensor("pred_polys_", [BLOC, NP, 2], F32, kind="ExternalInput")
    gt = nc.dram_tensor("gt_polys", [BLOC, NG, 2], F32, kind="ExternalInput")
    kmask = nc.dram_tensor("keyPointsMask", [BLOC, NG], F32, kind="ExternalInput")
    lhsQ_d = nc.dram_tensor("lhsQ", [BLOC, K8, NP], BF16, kind="ExternalInput")
    rhsQ_d = nc.dram_tensor("rhsQ", [BLOC, K8, NG], BF16, kind="ExternalInput")
    lhsR_d = nc.dram_tensor("lhsR", [BLOC, 10, NP], BF16, kind="ExternalInput")
    rhsR_d = nc.dram_tensor("rhsR", [BLOC, 10, NG], BF16, kind="ExternalInput")
    len2_d = nc.dram_tensor("len2_b", [BLOC, 128, NG], F32, kind="ExternalInput")
    gtl8_d = nc.dram_tensor("gtl8", [BLOC, K8, NG], BF16, kind="ExternalInput")
    prhs8_d = nc.dram_tensor("prhs8", [BLOC, K8, NP], BF16, kind="ExternalInput")
    uneg_d = nc.dram_tensor("uneg", [BLOC, 128, NCH], F32, kind="ExternalInput")
    # per-batch gather tables (offset-0 requirement for indirect DMA)
    itabws = [nc.dram_tensor(f"itabw{b_}", [NG, T * 2], F32, kind="ExternalInput")
              for b_ in range(BLOC)]
    ptabs = [nc.dram_tensor(f"ptab{b_}", [NP, 2], F32, kind="ExternalInput")
             for b_ in range(BLOC)]
    out = nc.dram_tensor("out", [128, 12], F32, kind="ExternalOutput")

    with TileContext(nc) as tc:
        with (
            tc.tile_pool(name="const", bufs=1) as cpool,
            tc.tile_pool(name="bat", bufs=2) as bat,
            tc.tile_pool(name="drain", bufs=3) as drp,
            tc.tile_pool(name="mrg", bufs=3) as mrg,
            tc.tile_pool(name="small", bufs=2) as small,
            tc.tile_pool(name="psA", bufs=3, space="PSUM") as psap,
            tc.tile_pool(name="psT", bufs=2, space="PSUM") as pstp,
        ):
            res = cpool.tile([128, 12], F32)
            nc.vector.memset(res[:], 0.0)
            c9 = cpool.tile([128, 1], F32)
            nc.vector.memset(c9[:], 9.0)
            candC = cpool.tile([128, BLOC, NCH, T, 2], F32)
            npredC = cpool.tile([128, BLOC, NCH, 2], F32)
            pxyC = cpool.tile([128, BLOC, NCH, 2], F32)
            pred2C = cpool.tile([128, BLOC, NCH, 2], F32)
            gtC = cpool.tile([128, BLOC, NCH, 2], F32)
            maskC = cpool.tile([128, BLOC, NCH], F32)
            uneg_sb = cpool.tile([128, BLOC, NCH], F32)
            nc.sync.dma_start(out=uneg_sb[:],
                              in_=uneg_d[:].rearrange("b p c -> p b c"))

            for b_ in range(BLOC):
                # ---------------- batch-level loads ----------------
                lhsQ = bat.tile([K8, NP], BF16, tag="lhsQ")
                nc.sync.dma_start(out=lhsQ[:], in_=lhsQ_d[b_])
                rhsQ = bat.tile([K8, NG], BF16, tag="rhsQ")
                nc.sync.dma_start(out=rhsQ[:], in_=rhsQ_d[b_])
                lhsR = bat.tile([10, NP], BF16, tag="lhsR")
                nc.sync.dma_start(out=lhsR[:], in_=lhsR_d[b_])
                rhsR = bat.tile([10, NG], BF16, tag="rhsR")
                nc.sync.dma_start(out=rhsR[:], in_=rhsR_d[b_])
                len2_b = bat.tile([128, NG], F32, tag="len2_b")
                nc.sync.dma_start(out=len2_b[:], in_=len2_d[b_])
                gtl8 = bat.tile([K8, NG], BF16, tag="gtl8")
                nc.scalar.dma_start(out=gtl8[:], in_=gtl8_d[b_])
                prhs8 = bat.tile([K8, NP], BF16, tag="prhs8")
                nc.scalar.dma_start(out=prhs8[:], in_=prhs8_d[b_])
                nc.sync.dma_start(
                    out=pxyC[:, b_],
                    in_=ini[b_][:].rearrange("(m p) c -> p m c", m=NCH))
                nc.sync.dma_start(
                    out=pred2C[:, b_],
                    in_=pred2[b_][:].rearrange("(m p) c -> p m c", m=NCH))
                nc.sync.dma_start(
                    out=gtC[:, b_], in_=gt[b_][:].rearrange("(m p) c -> p m c", m=NCH))
                nc.sync.dma_start(
                    out=maskC[:, b_],
                    in_=kmask[b_][:].rearrange("(c p) -> p c", p=128))

                # ---------------- pred2gt (point-to-segment projection) ------
                # Q = w.s in PSUM, R = -|w|^2 in PSUM (w = p - g_{i-1}).
                # t* = clamp(round(10 Q / len2), 0, 9) via Relu-chain on scalar
                # vkey = R + t*(0.2 Q - (len2/100) t*) = -d^2(t*)  (fp32)
                for m in range(NCH):
                    sl = slice(128 * m, 128 * (m + 1))
                    psQ = psap.tile([128, NG], F32, tag="psQ")
                    nc.tensor.matmul(psQ[:], lhsT=lhsQ[:, sl], rhs=rhsQ[:],
                                     start=True, stop=True)
                    psR = psap.tile([128, NG], F32, tag="psR")
                    nc.tensor.matmul(psR[:], lhsT=lhsR[:, sl], rhs=rhsR[:],
                                     start=True, stop=True)
                    # psQ holds z = 10*(w.s)/len2 directly (inv10 folded into
                    # rhsQ host-side).  clamp+round on scalar, polynomial on
                    # vector: vkey = R + len2*(0.02*z*t - 0.01*t^2) = -d^2(t*)
                    r1 = drp.tile([128, NG], F32, tag="r1")
                    nc.scalar.activation(out=r1[:], in_=psQ[:], func=AF.Relu)
                    r2 = drp.tile([128, NG], F32, tag="r2")
                    nc.scalar.activation(out=r2[:], in_=r1[:], func=AF.Relu,
                                         bias=c9[:, 0:1], scale=-1.0)
                    ym = drp.tile([128, NG], F32, tag="ym")
                    nc.scalar.activation(out=ym[:], in_=r2[:], func=AF.Copy,
                                         bias=8388617.0, scale=-1.0)
                    ts_ = drp.tile([128, NG], BF16, tag="ts_")
                    nc.scalar.activation(out=ts_[:], in_=ym[:], func=AF.Copy,
                                         bias=-8388608.0)
                    u2s = drp.tile([128, NG], F32, tag="u2s")
                    nc.scalar.activation(out=u2s[:], in_=ts_[:], func=AF.Square,
                                         scale=0.1)
                    t1 = mrg.tile([128, NG], F32, tag="t1")
                    nc.vector.tensor_tensor(out=t1[:], in0=psQ[:], in1=ts_[:],
                                            op=ALU.mult)
                    t3 = mrg.tile([128, NG], F32, tag="t3")
                    nc.vector.scalar_tensor_tensor(out=t3[:], in0=t1[:], scalar=0.02,
                                                   in1=u2s[:], op0=ALU.mult,
                                                   op1=ALU.subtract)
                    t4 = mrg.tile([128, NG], F32, tag="t4")
                    nc.vector.tensor_tensor(out=t4[:], in0=t3[:], in1=len2_b[:],
                                            op=ALU.mult)
                    vkey = mrg.tile([128, NG], F32, tag="vkey")
                    nc.vector.scalar_tensor_tensor(out=vkey[:], in0=psR[:], scalar=1.0,
                                                   in1=t4[:], op0=ALU.mult,
                                                   op1=ALU.add)
                    mx1 = small.tile([128, 1], F32, tag="mx1")
                    nc.vector.tensor_reduce(out=mx1[:], in_=vkey[:], axis=AX.X,
                                            op=ALU.max)
                    i8 = small.tile([128, 8], U32, tag="i8")
                    nc.vector.max_index(out=i8[:], in_max=mx1[:].to_broadcast([128, 8]),
                                        in_values=vkey[:])
                    nc.gpsimd.indirect_dma_start(
                        out=candC[:, b_, m].rearrange("p t c -> p (t c)"),
                        out_offset=None, in_=itabws[b_][:],
                        in_offset=IndirectOffsetOnAxis(ap=i8[:, 0:1], axis=0))

                # ---------------- gt2pred ----------------
                for c in range(NCH):
                    sl = slice(128 * c, 128 * (c + 1))
                    ps2 = pstp.tile([128, NP], F32, tag="psT")
                    nc.tensor.matmul(ps2[:], lhsT=gtl8[:, sl], rhs=prhs8[:],
                                     start=True, stop=True)
                    key2 = drp.tile([128, NP], BF16, tag="key2")
                    nc.scalar.activation(out=key2[:], in_=ps2[:], func=AF.Identity,
                                         bias=uneg_sb[:, b_, c:c + 1])
                    mxb1 = small.tile([128, 1], BF16, tag="mxb1")
                    nc.vector.tensor_reduce(out=mxb1[:], in_=key2[:], axis=AX.X,
                                            op=ALU.max)
                    ixb = small.tile([128, 8], U32, tag="ixb")
                    nc.vector.max_index(out=ixb[:],
                                        in_max=mxb1[:].to_broadcast([128, 8]),
                                        in_values=key2[:])
                    nc.gpsimd.indirect_dma_start(
                        out=npredC[:, b_, c, :], out_offset=None,
                        in_=ptabs[b_][:],
                        in_offset=IndirectOffsetOnAxis(ap=ixb[:, 0:1], axis=0))

            # ---------------- core-level refine + loss tails ----------------
            SH4 = [128, BLOC, NCH, T]
            dx = small.tile([128, BLOC, NCH, T], F32, tag="dx")
            dy = small.tile([128, BLOC, NCH, T], F32, tag="dy")
            nc.vector.tensor_tensor(
                out=dx[:], in0=candC[:, :, :, :, 0],
                in1=pxyC[:, :, :, 0:1].to_broadcast(SH4), op=ALU.subtract)
            nc.vector.tensor_tensor(
                out=dy[:], in0=candC[:, :, :, :, 1],
                in1=pxyC[:, :, :, 1:2].to_broadcast(SH4), op=ALU.subtract)
            sqx = small.tile([128, BLOC, NCH, T], F32, tag="sqx")
            sqy = small.tile([128, BLOC, NCH, T], F32, tag="sqy")
            dall = small.tile([128, BLOC, NCH, T], F32, tag="dall")
            nc.vector.tensor_tensor(out=sqx[:], in0=dx[:], in1=dx[:], op=ALU.mult)
            nc.vector.tensor_tensor(out=sqy[:], in0=dy[:], in1=dy[:], op=ALU.mult)
            nc.vector.tensor_tensor(out=dall[:], in0=sqx[:], in1=sqy[:], op=ALU.add)
            dmin = small.tile([128, BLOC, NCH], F32, tag="dmin")
            nc.vector.tensor_reduce(out=dmin[:], in_=dall[:], axis=AX.X, op=ALU.min)
            sel = small.tile([128, BLOC, NCH, T], F32, tag="sel")
            nc.vector.tensor_tensor(
                out=sel[:], in0=dall[:],
                in1=dmin[:].unsqueeze(3).to_broadcast(SH4), op=ALU.is_equal)
            selx = small.tile([128, BLOC, NCH, T], F32, tag="selx")
            sely = small.tile([128, BLOC, NCH, T], F32, tag="sely")
            nc.vector.tensor_tensor(out=selx[:], in0=sel[:],
                                    in1=candC[:, :, :, :, 0], op=ALU.mult)
            nc.vector.tensor_tensor(out=sely[:], in0=sel[:],
                                    in1=candC[:, :, :, :, 1], op=ALU.mult)
            nx = small.tile([128, BLOC, NCH], F32, tag="nx")
            ny = small.tile([128, BLOC, NCH], F32, tag="ny")
            nc.vector.tensor_reduce(out=nx[:], in_=selx[:], axis=AX.X, op=ALU.add)
            nc.vector.tensor_reduce(out=ny[:], in_=sely[:], axis=AX.X, op=ALU.add)
            df = small.tile([128, BLOC, NCH, 2], F32, tag="df")
            nc.vector.tensor_tensor(out=df[:, :, :, 0], in0=pred2C[:, :, :, 0],
                                    in1=nx[:], op=ALU.subtract)
            nc.vector.tensor_tensor(out=df[:, :, :, 1], in0=pred2C[:, :, :, 1],
                                    in1=ny[:], op=ALU.subtract)
            nc.vector.tensor_reduce(out=res[:, 0:BLOC], in_=df[:], axis=AX.XY,
                                    op=ALU.add, apply_absolute_value=True)

            md = small.tile([128, BLOC, NCH, 2], F32, tag="md")
            nc.vector.tensor_tensor(out=md[:], in0=npredC[:], in1=gtC[:],
                                    op=ALU.subtract)
            sabs = small.tile([128, BLOC, NCH], F32, tag="sabs")
            nc.vector.tensor_reduce(out=sabs[:], in_=md[:], axis=AX.X,
                                    op=ALU.add, apply_absolute_value=True)
            smask = small.tile([128, BLOC, NCH], F32, tag="smask")
            nc.vector.tensor_tensor(out=smask[:], in0=sabs[:], in1=maskC[:],
                                    op=ALU.mult)
            nc.vector.tensor_reduce(out=res[:, 4:4 + BLOC], in_=smask[:],
                                    axis=AX.X, op=ALU.add)
            nc.vector.tensor_reduce(out=res[:, 8:8 + BLOC], in_=maskC[:],
                                    axis=AX.X, op=ALU.add)

            nc.sync.dma_start(out=out[:], in_=res[:])

    nc.compile()
    return nc


_NC_CACHE = None


def _get_nc():
    global _NC_CACHE
    if _NC_CACHE is None:
        _NC_CACHE = build_nc()
    return _NC_CACHE


def make_in_maps(ini_pred_poly, pred_polys_, gt_polys, keyPointsMask):
    in_maps = []
    for i in range(NCORES):
        s = slice(BLOC * i, BLOC * (i + 1))
        ini = np.ascontiguousarray(ini_pred_poly[s], dtype=np.float32)
        p2 = np.ascontiguousarray(pred_polys_[s], dtype=np.float32)
        gp = np.ascontiguousarray(gt_polys[s], dtype=np.float32)
        km = np.ascontiguousarray(keyPointsMask[s], dtype=np.float32)
        hp = host_prep(ini, gp)
        im = {
            "ini_pred_poly": ini,
            "pred_polys_": p2,
            "gt_polys": gp,
            "keyPointsMask": km,
            "lhsQ": hp["lhsQ"],
            "rhsQ": hp["rhsQ"],
            "lhsR": hp["lhsR"],
            "rhsR": hp["rhsR"],
            "len2_b": hp["len2_b"],
            "gtl8": hp["gtl8"],
            "prhs8": hp["prhs8"],
            "uneg": hp["uneg"],
        }
        for b_ in range(BLOC):
            im[f"itabw{b_}"] = np.ascontiguousarray(hp["itabw"][b_])
            im[f"ptab{b_}"] = np.ascontiguousarray(p2[b_])
        in_maps.append(im)
    return in_maps


def combine_outputs(outs):
    """outs: list of [128, 12] per-core partial sums -> scalar loss (float32)."""
    acc = np.zeros(12, dtype=np.float64)
    for o in outs:
        acc += o.astype(np.float64).sum(axis=0)
    s_p2g = acc[0:4].sum()          # sum |pred_polys_ - nearest_gt|
    s_g2p = acc[4:8].sum()          # sum mask * |nearest_pred - gt|
    s_msk = 2.0 * acc[8:12].sum()   # sum of broadcast mask
    loss_pred2gt = s_p2g / (B * NP * 2)
    loss = (s_g2p / (s_msk + 1.0) + loss_pred2gt) / 2.0
    return np.float32(loss)


def kernel(ini_pred_poly, pred_polys_, gt_polys, keyPointsMask):
    nc = _get_nc()
    in_maps = make_in_maps(ini_pred_poly, pred_polys_, gt_polys, keyPointsMask)
    r = run_bass_kernel_spmd(nc, in_maps, list(range(NCORES)))
    return combine_outputs([r.results[i]["out"] for i in range(NCORES)])


if __name__ == "__main__":
    import reference

    inputs = {k: np.asarray(v) for k, v in reference.setup_inputs().items()}
    got = kernel(**inputs)
    print("kernel loss:", got)


# revision 30
# speedup vs baseline: 1.2181x; 1.2181x over previous
"""Trainium2 Bass kernel for nn_DMLoss_61942018343083 (Chamfer-style polygon
matching loss, retrieval_knn).

Sharding: data-parallel over batch B=32 across 8 NeuronCores (4 batches/core).
Each core computes partial sums into a [128, 12] output tile; the host combines
them into the scalar loss.

v2 design (vs the fp32-matmul v1):

pred2gt (argmin over 5120 interp points for each of 512 preds):
  * Ranking key v[p, (t,i)] = -d^2(p, interp(t,i)) + |p-256|^2 computed on the
    PE as a K=14 bf16 matmul per (pred-chunk, t): coordinates are recentered by
    -256 and split hi/lo into bf16 pairs (p ~ p_hi + p_lo), so each product
    p*r = p_hi*r_hi + p_hi*r_lo + p_lo*r_hi is exact to ~1 unit (lo*lo
    dropped).  bf16 matmuls run at 1 cycle/column vs fp32's 4.
  * All 14-row operand blocks are HOST-PREPARED (numpy) and DMA'd in; the
    3-per-tile packing at base partitions 0/32/64 satisfies the PE constraint
    that lhsT/rhs share a base partition in {0,32,64}.
  * t=0..5 accumulate in a 6-bank PSUM tile, reduced with one vector
    tensor_reduce(max) over a strided [128, 512, 6] view; t=6..9 drain via
    scalar ACTIVATE(Identity, bias=-|p|^2) to self-scaled bf16 and merge with
    3 vector TT(max) ops.  The [128,5120] key is never materialized.
  * MAX8/FIND_INDEX8 on the final [128,512] column-max give the best segment
    i* per pred; ONE indirect DMA per chunk gathers that segment's 10 interp
    points (host-prepped i-major table [512, 10*2]); exact fp32 refine over
    the 10 candidates picks the true nearest (CPU-sim: rel err 1.4e-4).

gt2pred (argmin over 512 preds for each of 512 gts):
  * Same trick, orientation flipped: K=8 bf16 matmul per gt-chunk
    (psum = 2*g.p - |p|^2), scalar drain with bias -|g|^2 -> bf16 -d^2 key,
    top-1 via MAX8/FIND_INDEX8, gather pred row, masked abs-diff partials.
"""

import os
import sys

for _p in ("/opt/trn_rl_repo", "/root/.axon_site/_ro/trn_rl_repo"):
    if os.path.isdir(_p) and _p not in sys.path:
        sys.path.insert(0, _p)

import numpy as np
import ml_dtypes

bfloat16 = ml_dtypes.bfloat16

import concourse.bass as bass
import concourse.bacc as bacc
import concourse.mybir as mybir
from concourse.bass import IndirectOffsetOnAxis
from concourse.bass_utils import run_bass_kernel_spmd
from concourse.tile import TileContext

F32 = mybir.dt.float32
BF16 = mybir.dt.bfloat16
U32 = mybir.dt.uint32
AF = mybir.ActivationFunctionType
ALU = mybir.AluOpType
AX = mybir.AxisListType

B, NP, NG, T = 32, 512, 512, 10
NCORES = 8
BLOC = B // NCORES          # 4 batches per core
NCH = NP // 128             # 4 chunks of 128 preds / 128 gts
CEN = np.float32(256.0)     # recentering shift
NRED = 6                    # t-banks reduced on vector; T-NRED drained on scalar
K14 = 14                    # pred2gt contraction rows
K8 = 8                      # gt2pred contraction rows


def _split_hi_lo(x):
    x = np.asarray(x, dtype=np.float32)
    hi = x.astype(bfloat16)
    lo = (x - hi.astype(np.float32)).astype(bfloat16)
    return hi, lo


def host_prep(ini_pred_poly, gt_polys):
    """Build all matmul operands / tables for one core's BLOC batches."""
    f = np.float32
    a = (np.arange(T, dtype=np.float32) / f(T)).astype(np.float32)   # t/10
    b_ = (f(1.0) - a).astype(np.float32)

    ini = np.asarray(ini_pred_poly, dtype=np.float32)   # [BLOC, NP, 2]
    gt = np.asarray(gt_polys, dtype=np.float32)         # [BLOC, NG, 2]
    gtr = np.roll(gt, 1, axis=1)

    pc = ini - CEN
    gc = gt - CEN
    gcr = np.roll(gc, 1, axis=1)

    pxh, pxl = _split_hi_lo(pc[:, :, 0])
    pyh, pyl = _split_hi_lo(pc[:, :, 1])
    m1 = np.full_like(pxh, -1.0)

    # ---- pred2gt projection operands ----
    # Q[p,i] = (p - g_{i-1}) . s_i    (s = g_i - g_{i-1})
    # R[p,i] = -|p - g_{i-1}|^2  =  2 p.g_ - |g_|^2 - |p|^2   (centered coords)
    pp = (pc * pc).sum(-1).astype(np.float32)               # |p-256|^2 [BLOC, NP]
    s = (gc - gcr).astype(np.float32)
    len2 = (s * s).sum(-1).astype(np.float32)
    inv10 = np.where(len2 > 1e-6, (f(10.0) / len2).astype(np.float32),
                     np.float32(0.0)).astype(np.float32)
    negalpha = (-(len2 / f(100.0))).astype(np.float32)
    gs_ = (gcr * s).sum(-1).astype(np.float32)
    ug_ = (gcr * gcr).sum(-1).astype(np.float32)

    one = np.ones_like(pxh, dtype=np.float32)
    sxh, sxl = _split_hi_lo((s[:, :, 0] * inv10).astype(np.float32))
    syh, syl = _split_hi_lo((s[:, :, 1] * inv10).astype(np.float32))
    gsh, gsl = _split_hi_lo((gs_ * inv10).astype(np.float32))
    lhsQ = np.stack([pxh, pxh, pxl, pyh, pyh, pyl, m1, m1],
                    axis=1).astype(bfloat16)                  # [BLOC, 8, NP]
    rhsQ = np.stack([sxh, sxl, sxh, syh, syl, syh, gsh, gsl],
                    axis=1).astype(bfloat16)                  # [BLOC, 8, NG]

    g2rxh, g2rxl = _split_hi_lo(f(2.0) * gcr[:, :, 0])
    g2ryh, g2ryl = _split_hi_lo(f(2.0) * gcr[:, :, 1])
    ugh, ugl = _split_hi_lo(ug_)
    npph, nppl = _split_hi_lo(-pp)
    oneb = one.astype(bfloat16)
    lhsR = np.stack([pxh, pxh, pxl, pyh, pyh, pyl, m1, m1, npph, nppl],
                    axis=1).astype(bfloat16)                  # [BLOC, 10, NP]
    rhsR = np.stack([g2rxh, g2rxl, g2rxh, g2ryh, g2ryl, g2ryh, ugh, ugl,
                     oneb, oneb], axis=1).astype(bfloat16)    # [BLOC, 10, NG]
    # per-column broadcast tile (replicated across 128 partitions host-side)
    len2_b = np.broadcast_to(len2[:, None, :], (BLOC, 128, NG)).copy()

    # ---- interp table, i-major wide rows [BLOC, NG, T*2] fp32 (bit-exact ref math)
    itabw = np.empty((BLOC, NG, T, 2), dtype=np.float32)
    for t in range(T):
        itabw[:, :, t, :] = (gt * a[t]).astype(np.float32) + (gtr * (f(1.0) - a[t])).astype(np.float32)
    itabw = itabw.reshape(BLOC, NG, T * 2)

    # ---- gt2pred: lhs rows [g2xh,g2xh,g2xl,g2yh,g2yh,g2yl,m1,m1] over gts
    g2xh, g2xl = _split_hi_lo(f(2.0) * gc[:, :, 0])
    g2yh, g2yl = _split_hi_lo(f(2.0) * gc[:, :, 1])
    m1g = np.full_like(g2xh, -1.0)
    gtl8 = np.stack([g2xh, g2xh, g2xl, g2yh, g2yh, g2yl, m1g, m1g],
                    axis=1).astype(bfloat16)                 # [BLOC, 8, NG]
    pph, ppl = _split_hi_lo(pp)
    prhs8 = np.stack([pxh, pxl, pxh, pyh, pyl, pyh, pph, ppl],
                     axis=1).astype(bfloat16)                # [BLOC, 8, NP]
    ug = (gc * gc).sum(-1).astype(np.float32)                # |g-256|^2 [BLOC, NG]
    uneg = (-ug).reshape(BLOC, NCH, 128).transpose(0, 2, 1).copy()

    return dict(lhsQ=lhsQ, rhsQ=rhsQ, lhsR=lhsR, rhsR=rhsR, len2_b=len2_b,
                itabw=itabw, gtl8=gtl8, prhs8=prhs8, uneg=uneg)


def build_nc():
    nc = bacc.Bacc()

    ini = nc.dram_tensor("ini_pred_poly", [BLOC, NP, 2], F32, kind="ExternalInput")
    pred2 = nc.dram_tensor("pred_polys_", [BLOC, NP, 2], F32, kind="ExternalInput")
    gt = nc.dram_tensor("gt_polys", [BLOC, NG, 2], F32, kind="ExternalInput")
    kmask = nc.dram_tensor("keyPointsMask", [BLOC, NG], F32, kind="ExternalInput")
    lhsQ_d = nc.dram_tensor("lhsQ", [BLOC, K8, NP], BF16, kind="ExternalInput")
    rhsQ_d = nc.dram_tensor("rhsQ", [BLOC, K8, NG], BF16, kind="ExternalInput")
    lhsR_d = nc.dram_tensor("lhsR", [BLOC, 10, NP], BF16, kind="ExternalInput")
    rhsR_d = nc.dram_tensor("rhsR", [BLOC, 10, NG], BF16, kind="ExternalInput")
    len2_d = nc.dram_tensor("len2_b", [BLOC, 128, NG], F32, kind="ExternalInput")
    gtl8_d = nc.dram_tensor("gtl8", [BLOC, K8, NG], BF16, kind="ExternalInput")
    prhs8_d = nc.dram_tensor("prhs8", [BLOC, K8, NP], BF16, kind="ExternalInput")
    uneg_d = nc.dram_tensor("uneg", [BLOC, 128, NCH], F32, kind="ExternalInput")
    # per-batch gather tables (offset-0 requirement for indirect DMA)
    itabws = [nc.dram_tensor(f"itabw{b_}", [NG, T * 2], F32, kind="ExternalInput")
              for b_ in range(BLOC)]
    ptabs = [nc.dram_tensor(f"ptab{b_}", [NP, 2], F32, kind="ExternalInput")
             for b_ in range(BLOC)]
    out = nc.dram_tensor("out", [128, 12], F32, kind="ExternalOutput")

    with TileContext(nc) as tc:
        with (
            tc.tile_pool(name="const", bufs=1) as cpool,
            tc.tile_pool(name="bat", bufs=2) as bat,
            tc.tile_pool(name="drain", bufs=3) as drp,
            tc.tile_pool(name="mrg", bufs=3) as mrg,
            tc.tile_pool(name="small", bufs=2) as small,
            tc.tile_pool(name="psA", bufs=3, space="PSUM") as psap,
            tc.tile_pool(name="psT", bufs=2, space="PSUM") as pstp,
        ):
            res = cpool.tile([128, 12], F32)
            nc.vector.memset(res[:], 0.0)
            c9 = cpool.tile([128, 1], F32)
            nc.vector.memset(c9[:], 9.0)
            candC = cpool.tile([128, BLOC, NCH, T, 2], F32)
            npredC = cpool.tile([128, BLOC, NCH, 2], F32)
            pxyC = cpool.tile([128, BLOC, NCH, 2], F32)
            pred2C = cpool.tile([128, BLOC, NCH, 2], F32)
            gtC = cpool.tile([128, BLOC, NCH, 2], F32)
            maskC = cpool.tile([128, BLOC, NCH], F32)
            uneg_sb = cpool.tile([128, BLOC, NCH], F32)
            nc.sync.dma_start(out=uneg_sb[:],
                              in_=uneg_d[:].rearrange("b p c -> p b c"))

            for b_ in range(BLOC):
                # ---------------- batch-level loads ----------------
                lhsQ = bat.tile([K8, NP], BF16, tag="lhsQ")
                nc.sync.dma_start(out=lhsQ[:], in_=lhsQ_d[b_])
                rhsQ = bat.tile([K8, NG], BF16, tag="rhsQ")
                nc.sync.dma_start(out=rhsQ[:], in_=rhsQ_d[b_])
                lhsR = bat.tile([10, NP], BF16, tag="lhsR")
                nc.sync.dma_start(out=lhsR[:], in_=lhsR_d[b_])
                rhsR = bat.tile([10, NG], BF16, tag="rhsR")
                nc.sync.dma_start(out=rhsR[:], in_=rhsR_d[b_])
                len2_b = bat.tile([128, NG], F32, tag="len2_b")
                nc.sync.dma_start(out=len2_b[:], in_=len2_d[b_])
                gtl8 = bat.tile([K8, NG], BF16, tag="gtl8")
                nc.scalar.dma_start(out=gtl8[:], in_=gtl8_d[b_])
                prhs8 = bat.tile([K8, NP], BF16, tag="prhs8")
                nc.scalar.dma_start(out=prhs8[:], in_=prhs8_d[b_])
                nc.sync.dma_start(
                    out=pxyC[:, b_],
                    in_=ini[b_][:].rearrange("(m p) c -> p m c", m=NCH))
                nc.sync.dma_start(
                    out=pred2C[:, b_],
                    in_=pred2[b_][:].rearrange("(m p) c -> p m c", m=NCH))
                nc.sync.dma_start(
                    out=gtC[:, b_], in_=gt[b_][:].rearrange("(m p) c -> p m c", m=NCH))
                nc.sync.dma_start(
                    out=maskC[:, b_],
                    in_=kmask[b_][:].rearrange("(c p) -> p c", p=128))

                # ---------------- pred2gt (point-to-segment projection) ------
                # Q = w.s in PSUM, R = -|w|^2 in PSUM (w = p - g_{i-1}).
                # t* = clamp(round(10 Q / len2), 0, 9) via Relu-chain on scalar
                # vkey = R + t*(0.2 Q - (len2/100) t*) = -d^2(t*)  (fp32)
                for m in range(NCH):
                    sl = slice(128 * m, 128 * (m + 1))
                    psQ = psap.tile([128, NG], F32, tag="psQ")
                    nc.tensor.matmul(psQ[:], lhsT=lhsQ[:, sl], rhs=rhsQ[:],
                                     start=True, stop=True)
                    psR = psap.tile([128, NG], F32, tag="psR")
                    nc.tensor.matmul(psR[:], lhsT=lhsR[:, sl], rhs=rhsR[:],
                                     start=True, stop=True)
                    # psQ holds z = 10*(w.s)/len2 directly (inv10 folded into
                    # rhsQ host-side).  clamp+round on scalar, polynomial on
                    # vector: vkey = R + len2*(0.02*z*t - 0.01*t^2) = -d^2(t*)
                    r1 = drp.tile([128, NG], F32, tag="r1")
                    nc.scalar.activation(out=r1[:], in_=psQ[:], func=AF.Relu)
                    r2 = drp.tile([128, NG], F32, tag="r2")
                    nc.scalar.activation(out=r2[:], in_=r1[:], func=AF.Relu,
                                         bias=c9[:, 0:1], scale=-1.0)
                    ym = drp.tile([128, NG], F32, tag="ym")
                    nc.scalar.activation(out=ym[:], in_=r2[:], func=AF.Copy,
                                         bias=8388617.0, scale=-1.0)
                    ts_ = drp.tile([128, NG], BF16, tag="ts_")
                    nc.scalar.activation(out=ts_[:], in_=ym[:], func=AF.Copy,
                                         bias=-8388608.0)
                    u2s = drp.tile([128, NG], F32, tag="u2s")
                    nc.scalar.activation(out=u2s[:], in_=ts_[:], func=AF.Square,
                                         scale=0.1)
                    t1 = mrg.tile([128, NG], F32, tag="t1")
                    nc.vector.tensor_tensor(out=t1[:], in0=psQ[:], in1=ts_[:],
                                            op=ALU.mult)
                    t3 = mrg.tile([128, NG], F32, tag="t3")
                    nc.vector.scalar_tensor_tensor(out=t3[:], in0=t1[:], scalar=0.02,
                                                   in1=u2s[:], op0=ALU.mult,
                                                   op1=ALU.subtract)
                    t4 = mrg.tile([128, NG], F32, tag="t4")
                    nc.vector.tensor_tensor(out=t4[:], in0=t3[:], in1=len2_b[:],
                                            op=ALU.mult)
                    vkey = mrg.tile([128, NG], F32, tag="vkey")
                    nc.vector.scalar_tensor_tensor(out=vkey[:], in0=psR[:], scalar=1.0,
                                                   in1=t4[:], op0=ALU.mult,
                                                   op1=ALU.add)
                    mx1 = small.tile([128, 1], F32, tag="mx1")
                    nc.vector.tensor_reduce(out=mx1[:], in_=vkey[:], axis=AX.X,
                                            op=ALU.max)
                    i8 = small.tile([128, 8], U32, tag="i8")
                    nc.vector.max_index(out=i8[:], in_max=mx1[:].to_broadcast([128, 8]),
                                        in_values=vkey[:])
                    nc.gpsimd.indirect_dma_start(
                        out=candC[:, b_, m].rearrange("p t c -> p (t c)"),
                        out_offset=None, in_=itabws[b_][:],
                        in_offset=IndirectOffsetOnAxis(ap=i8[:, 0:1], axis=0))

                # ---------------- gt2pred ----------------
                for c in range(NCH):
                    sl = slice(128 * c, 128 * (c + 1))
                    ps2 = pstp.tile([128, NP], F32, tag="psT")
                    nc.tensor.matmul(ps2[:], lhsT=gtl8[:, sl], rhs=prhs8[:],
                                     start=True, stop=True)
                    key2 = drp.tile([128, NP], BF16, tag="key2")
                    nc.scalar.activation(out=key2[:], in_=ps2[:], func=AF.Identity,
                                         bias=uneg_sb[:, b_, c:c + 1])
                    mxb1 = small.tile([128, 1], BF16, tag="mxb1")
                    nc.vector.tensor_reduce(out=mxb1[:], in_=key2[:], axis=AX.X,
                                            op=ALU.max)
                    ixb = small.tile([128, 8], U32, tag="ixb")
                    nc.vector.max_index(out=ixb[:],
                                        in_max=mxb1[:].to_broadcast([128, 8]),
                                        in_values=key2[:])
                    nc.gpsimd.indirect_dma_start(
                        out=npredC[:, b_, c, :], out_offset=None,
                        in_=ptabs[b_][:],
                        in_offset=IndirectOffsetOnAxis(ap=ixb[:, 0:1], axis=0))

            # ---------------- core-level refine + loss tails ----------------
            SH4 = [128, BLOC, NCH, T]
            dx = small.tile([128, BLOC, NCH, T], F32, tag="dx")
            dy = small.tile([128, BLOC, NCH, T], F32, tag="dy")
            nc.vector.tensor_tensor(
                out=dx[:], in0=candC[:, :, :, :, 0],
                in1=pxyC[:, :, :, 0:1].to_broadcast(SH4), op=ALU.subtract)
            nc.vector.tensor_tensor(
                out=dy[:], in0=candC[:, :, :, :, 1],
                in1=pxyC[:, :, :, 1:2].to_broadcast(SH4), op=ALU.subtract)
            sqx = small.tile([128, BLOC, NCH, T], F32, tag="sqx")
            sqy = small.tile([128, BLOC, NCH, T], F32, tag="sqy")
            dall = small.tile([128, BLOC, NCH, T], F32, tag="dall")
            nc.vector.tensor_tensor(out=sqx[:], in0=dx[:], in1=dx[:], op=ALU.mult)
            nc.vector.tensor_tensor(out=sqy[:], in0=dy[:], in1=dy[:], op=ALU.mult)
            nc.vector.tensor_tensor(out=dall[:], in0=sqx[:], in1=sqy[:], op=ALU.add)
            dmin = small.tile([128, BLOC, NCH], F32, tag="dmin")
            nc.vector.tensor_reduce(out=dmin[:], in_=dall[:], axis=AX.X, op=ALU.min)
            sel = small.tile([128, BLOC, NCH, T], F32, tag="sel")
            nc.vector.tensor_tensor(
                out=sel[:], in0=dall[:],
                in1=dmin[:].unsqueeze(3).to_broadcast(SH4), op=ALU.is_equal)
            selx = small.tile([128, BLOC, NCH, T], F32, tag="selx")
            sely = small.tile([128, BLOC, NCH, T], F32, tag="sely")
            nc.vector.tensor_tensor(out=selx[:], in0=sel[:],
                                    in1=candC[:, :, :, :, 0], op=ALU.mult)
            nc.vector.tensor_tensor(out=sely[:], in0=sel[:],
                                    in1=candC[:, :, :, :, 1], op=ALU.mult)
            nx = small.tile([128, BLOC, NCH], F32, tag="nx")
            ny = small.tile([128, BLOC, NCH], F32, tag="ny")
            nc.vector.tensor_reduce(out=nx[:], in_=selx[:], axis=AX.X, op=ALU.add)
            nc.vector.tensor_reduce(out=ny[:], in_=sely[:], axis=AX.X, op=ALU.add)
            df = small.tile([128, BLOC, NCH, 2], F32, tag="df")
            nc.vector.tensor_tensor(out=df[:, :, :, 0], in0=pred2C[:, :, :, 0],
                                    in1=nx[:], op=ALU.subtract)
            nc.vector.tensor_tensor(out=df[:, :, :, 1], in0=pred2C[:, :, :, 1],
                                    in1=ny[:], op=ALU.subtract)
            nc.vector.tensor_reduce(out=res[:, 0:BLOC], in_=df[:], axis=AX.XY,
                                    op=ALU.add, apply_absolute_value=True)

            md = small.tile([128, BLOC, NCH, 2], F32, tag="md")
            nc.vector.tensor_tensor(out=md[:], in0=npredC[:], in1=gtC[:],
                                    op=ALU.subtract)
            sabs = small.tile([128, BLOC, NCH], F32, tag="sabs")
            nc.vector.tensor_reduce(out=sabs[:], in_=md[:], axis=AX.X,
                                    op=ALU.add, apply_absolute_value=True)
            smask = small.tile([128, BLOC, NCH], F32, tag="smask")
            nc.vector.tensor_tensor(out=smask[:], in0=sabs[:], in1=maskC[:],
                                    op=ALU.mult)
            nc.vector.tensor_reduce(out=res[:, 4:4 + BLOC], in_=smask[:],
                                    axis=AX.X, op=ALU.add)
            nc.vector.tensor_reduce(out=res[:, 8:8 + BLOC], in_=maskC[:],
                                    axis=AX.X, op=ALU.add)

            nc.sync.dma_start(out=out[:], in_=res[:])

    nc.compile()
    return nc


_NC_CACHE = None


def _get_nc():
    global _NC_CACHE
    if _NC_CACHE is None:
        _NC_CACHE = build_nc()
    return _NC_CACHE


def make_in_maps(ini_pred_poly, pred_polys_, gt_polys, keyPointsMask):
    in_maps = []
    for i in range(NCORES):
        s = slice(BLOC * i, BLOC * (i + 1))
        ini = np.ascontiguousarray(ini_pred_poly[s], dtype=np.float32)
        p2 = np.ascontiguousarray(pred_polys_[s], dtype=np.float32)
        gp = np.ascontiguousarray(gt_polys[s], dtype=np.float32)
        km = np.ascontiguousarray(keyPointsMask[s], dtype=np.float32)
        hp = host_prep(ini, gp)
        im = {
            "ini_pred_poly": ini,
            "pred_polys_": p2,
            "gt_polys": gp,
            "keyPointsMask": km,
            "lhsQ": hp["lhsQ"],
            "rhsQ": hp["rhsQ"],
            "lhsR": hp["lhsR"],
            "rhsR": hp["rhsR"],
            "len2_b": hp["len2_b"],
            "gtl8": hp["gtl8"],
            "prhs8": hp["prhs8"],
            "uneg": hp["uneg"],
        }
        for b_ in range(BLOC):
            im[f"itabw{b_}"] = np.ascontiguousarray(hp["itabw"][b_])
            im[f"ptab{b_}"] = np.ascontiguousarray(p2[b_])
        in_maps.append(im)
    return in_maps


def combine_outputs(outs):
    """outs: list of [128, 12] per-core partial sums -> scalar loss (float32)."""
    acc = np.zeros(12, dtype=np.float64)
    for o in outs:
        acc += o.astype(np.float64).sum(axis=0)
    s_p2g = acc[0:4].sum()          # sum |pred_polys_ - nearest_gt|
    s_g2p = acc[4:8].sum()          # sum mask * |nearest_pred - gt|
    s_msk = 2.0 * acc[8:12].sum()   # sum of broadcast mask
    loss_pred2gt = s_p2g / (B * NP * 2)
    loss = (s_g2p / (s_msk + 1.0) + loss_pred2gt) / 2.0
    return np.float32(loss)


def kernel(ini_pred_poly, pred_polys_, gt_polys, keyPointsMask):
    nc = _get_nc()
    in_maps = make_in_maps(ini_pred_poly, pred_polys_, gt_polys, keyPointsMask)
    r = run_bass_kernel_spmd(nc, in_maps, list(range(NCORES)))
    return combine_outputs([r.results[i]["out"] for i in range(NCORES)])


if __name__ == "__main__":
    import reference

    inputs = {k: np.asarray(v) for k, v in reference.setup_inputs().items()}
    got = kernel(**inputs)
    print("kernel loss:", got)
